# revision 1
# baseline (speedup 1.0000x reference)
"""Dcls3d (learnable-position dilated conv3d) Trainium2 kernel.

Reference computes:
  K = trilinear-scatter(weight, P) -> (64, 32, 5, 5, 5)
  out = conv3d(x, K, stride 1, pad 2) + bias     x: (2,32,16,32,32) -> out: (2,64,16,32,32)

Strategy (8 cores): shard (batch n in {0,1}) x (4 chunks of 4 output d-planes).
Each core runs an implicit-GEMM direct conv:
  - input slab (zero-padded on host) replicated 4x in SBUF, w-shifted by
    delta=0..3, giving a 128-partition (delta, ic) contraction axis.
  - for each of 25 (l, j) kernel-tap pairs: one matmul contracting
    (4 w-taps x 32 ic) = 128, M=64 out-channels, N=512 outputs, accumulating
    in PSUM; the i=4 leftover tap runs as a K=32 matmul off the delta-group.
  - bias added during PSUM->SBUF copyback; one 1MB store per core.
"""

import numpy as np

import concourse.bass as bass
import concourse.bacc as bacc
import concourse.mybir as mybir
from concourse.bass_utils import run_bass_kernel_spmd
from concourse.tile import TileContext

# ---- problem constants (hardcoded per contract) ----
N, IC, D, H, W = 2, 32, 16, 32, 32
OC = 64
KC = 16
PAD = 2
DP, HP, WP = D + 2 * PAD, H + 2 * PAD, W + 2 * PAD  # 20, 36, 36
DCHUNK = 4              # output d-planes per core
DSLAB = DCHUNK + 4      # input d-planes per core (halo 2 each side)
SLABF = DSLAB * HP * WP  # 8*36*36 = 10368
XS_COLS = SLABF + 4     # slack so the delta-shifted loads stay in bounds
NTAPS_LJ = 25
OUTF = DCHUNK * H * W   # 4096 outputs per (core, oc)

_NC_CACHE = {}


def _construct_K(weight, P):
    """Exact numpy port of reference.construct_kernel for ks=(5,5,5)."""
    Pp = P + np.float32(2.0)
    Pf = np.floor(Pp)
    R = Pp - Pf
    P1, P2, P3 = Pf[0], Pf[1], Pf[2]
    R1, R2, R3 = R[0], R[1], R[2]
    g = np.arange(5, dtype=P.dtype)[:, None, None, None]
    aL = (g == P1) * (1.0 - R1) + (g == P1 + 1.0) * R1
    aJ = (g == P3) * (1.0 - R3) + (g == P3 + 1.0) * R3
    aI = (g == P2) * (1.0 - R2) + (g == P2 + 1.0) * R2
    K = np.einsum("ock,lock,jock,iock->oclji", weight, aL, aJ, aI, optimize=True)
    return np.ascontiguousarray(K.astype(np.float32))


LJ_A = [lj for lj in range(NTAPS_LJ) if lj % 2 == 0]  # col-group 0 taps
LJ_B = [lj for lj in range(NTAPS_LJ) if lj % 2 == 1]  # col-group 1 taps
ROW_PACK = False  # leftover i=4 taps spread across PE row groups


def _build_nc_packed(mm="bf16"):
    """v1: col-group packed (2 taps concurrently on PE) + row-packed i=4."""
    key = ("v1", mm, ROW_PACK)
    if key in _NC_CACHE:
        return _NC_CACHE[key]
    f32 = mybir.dt.float32
    mdt = {"f32": f32, "bf16": mybir.dt.bfloat16}[mm]
    nc = bacc.Bacc()
    xs = nc.dram_tensor("xs", [IC, XS_COLS], mdt, kind="ExternalInput")
    kta = nc.dram_tensor("kta", [128, len(LJ_A) * OC], mdt, kind="ExternalInput")
    ktb = nc.dram_tensor("ktb", [128, len(LJ_B) * OC], mdt, kind="ExternalInput")
    ktd = nc.dram_tensor("ktd", [128, 5 * OC], mdt, kind="ExternalInput")
    ktj = nc.dram_tensor("ktj", [128, OC], mdt, kind="ExternalInput")
    kt5 = nc.dram_tensor("kt5", [IC, OC], mdt, kind="ExternalInput")
    bias = nc.dram_tensor("bias", [OC, 1], f32, kind="ExternalInput")
    out = nc.dram_tensor("out", [OC, OUTF], f32, kind="ExternalOutput")

    HALF = 6 * HP * WP  # six d-planes per xrep half
    with TileContext(nc) as tc:
        with (
            tc.tile_pool(name="const", bufs=1) as cpool,
            tc.tile_pool(name="psum", bufs=8, space="PSUM") as ppool,
        ):
            kta_sb = cpool.tile([128, len(LJ_A) * OC], mdt)
            nc.sync.dma_start(out=kta_sb, in_=kta[:, :])
            ktb_sb = cpool.tile([128, len(LJ_B) * OC], mdt)
            nc.sync.dma_start(out=ktb_sb, in_=ktb[:, :])
            ktd_sb = cpool.tile([128, 5 * OC], mdt)
            nc.sync.dma_start(out=ktd_sb, in_=ktd[:, :])
            ktj_sb = cpool.tile([128, OC], mdt)
            nc.sync.dma_start(out=ktj_sb, in_=ktj[:, :])
            kt5_sb = cpool.tile([IC, OC], mdt)
            nc.sync.dma_start(out=kt5_sb, in_=kt5[:, :])
            bias_sb = cpool.tile([OC, 1], f32)
            nc.sync.dma_start(out=bias_sb, in_=bias[:, :])
            # input slab split in two halves (planes 0-5 / 2-7) so out d=0,1
            # compute starts while the second half still loads
            xrepA = cpool.tile([128, HALF], mdt)
            xrepB = cpool.tile([128, HALF], mdt)
            for dl in range(4):
                nc.sync.dma_start(
                    out=xrepA[dl * IC : (dl + 1) * IC, :], in_=xs[:, dl : dl + HALF]
                )
            for dl in range(4):
                nc.sync.dma_start(
                    out=xrepB[dl * IC : (dl + 1) * IC, :],
                    in_=xs[:, 2 * HP * WP + dl : 2 * HP * WP + dl + HALF],
                )
            obufs = [cpool.tile([OC, H * W], f32, name=f"obuf{d}") for d in range(4)]

            # d-shifted replication for the i=4 taps: partition group
            # lam holds xs shifted by lam d-planes AND +4 in w, so one
            # K=128 matmul covers taps (l=lam, j, i=4) for lam=0..3.
            DWIN = 4 * HP * WP
            xrepD = cpool.tile([128, DWIN], mdt)
            for lam in range(4):
                o = lam * HP * WP + 4
                nc.sync.dma_start(
                    out=xrepD[lam * IC : (lam + 1) * IC, :], in_=xs[:, o : o + DWIN]
                )
            # h-row (j) shifted replication for taps (l=4, j=0..3, i=4):
            # partition group mu holds planes 4..7 shifted by mu rows and +4 w
            JWIN = 5040
            xrepJ = cpool.tile([128, JWIN], mdt)
            for mu in range(4):
                o = 4 * HP * WP + mu * WP + 4
                nc.sync.dma_start(
                    out=xrepJ[mu * IC : (mu + 1) * IC, :], in_=xs[:, o : o + JWIN]
                )

            xrepA_r = xrepA.rearrange("p (r w) -> p r w", w=WP)
            xrepB_r = xrepB.rearrange("p (r w) -> p r w", w=WP)
            xrepD_r = xrepD.rearrange("p (r w) -> p r w", w=WP)
            xrepJ_r = xrepJ.rearrange("p (r w) -> p r w", w=WP)

            def tile_geom(t):
                d, h0 = divmod(t, 2)
                h0 *= 16
                xr = xrepA_r if d < 2 else xrepB_r
                dbase = 0 if d < 2 else 2
                return d, h0, xr, dbase

            # pass 1: all w-packed taps (need only xrepA/xrepB) for all 8
            # tiles -- 8 psum banks accumulate concurrently, so the PE never
            # stalls on the later xrepD/xrepJ DMAs.
            pss = []
            for t in range(8):
                d, h0, xrep_r, dbase = tile_geom(t)
                ps = ppool.tile([128, 512], f32)
                pss.append(ps)
                for s in range(len(LJ_A)):
                    for grp, ljs, ktsb in ((0, LJ_A, kta_sb), (1, LJ_B, ktb_sb)):
                        if s >= len(ljs):
                            continue
                        lj = ljs[s]
                        l, j = divmod(lj, 5)
                        r = (d + l - dbase) * HP + h0 + j
                        nc.tensor.matmul(
                            ps[grp * 64 : grp * 64 + 64, :],
                            ktsb[:, s * OC : (s + 1) * OC],
                            xrep_r[:, r : r + 16, 0:W],
                            start=(s == 0),
                            stop=False,
                            skip_group_check=True,
                            tile_position=(0, grp * 64),
                        )
            # pass 2: i=4 closers off xrepD/xrepJ + corner single + epilogue
            for t in range(8):
                d, h0, xrep_r, dbase = tile_geom(t)
                ps = pss[t]
                for j in range(5):
                    grp = j % 2
                    nc.tensor.matmul(
                        ps[grp * 64 : grp * 64 + 64, :],
                        ktd_sb[:, j * OC : (j + 1) * OC],
                        xrepD_r[:, d * HP + h0 + j : d * HP + h0 + j + 16, 0:W],
                        start=False,
                        stop=False,
                        skip_group_check=True,
                        tile_position=(0, grp * 64),
                    )
                nc.tensor.matmul(
                    ps[64:128, :],
                    ktj_sb[:, :],
                    xrepJ_r[:, d * HP + h0 : d * HP + h0 + 16, 0:W],
                    start=False,
                    stop=True,
                    skip_group_check=True,
                    tile_position=(0, 64),
                )
                r45 = (d + 4 - dbase) * HP + h0 + 4  # tap (l=4, j=4)
                nc.tensor.matmul(
                    ps[0:64, :],
                    kt5_sb[0:IC, :],
                    xrep_r[0:IC, r45 : r45 + 16, 4 : 4 + W],
                    start=False,
                    stop=True,
                    skip_group_check=True,
                    tile_position=(0, 0),
                )
                oslice = obufs[d][:, (t % 2) * 512 : (t % 2) * 512 + 512]
                nc.vector.tensor_scalar_add(out=oslice, in0=ps[0:64, :], scalar1=bias_sb)
                nc.vector.tensor_tensor(
                    out=oslice, in0=ps[64:128, :], in1=oslice,
                    op=mybir.AluOpType.add,
                )
                if t % 2 == 1:
                    nc.sync.dma_start(
                        out=out[:, d * H * W : (d + 1) * H * W], in_=obufs[d]
                    )
    nc.finalize()
    _NC_CACHE[key] = nc
    return nc


def _build_nc(mm="bf16"):
    key = ("v0", mm)
    if key in _NC_CACHE:
        return _NC_CACHE[key]
    f32 = mybir.dt.float32
    mdt = {"f32": f32, "bf16": mybir.dt.bfloat16}[mm]
    nc = bacc.Bacc()
    xs = nc.dram_tensor("xs", [IC, XS_COLS], mdt, kind="ExternalInput")
    kt = nc.dram_tensor("kt", [128, NTAPS_LJ * OC], mdt, kind="ExternalInput")
    kt4 = nc.dram_tensor("kt4", [IC, NTAPS_LJ * OC], mdt, kind="ExternalInput")
    bias = nc.dram_tensor("bias", [OC, 1], f32, kind="ExternalInput")
    out = nc.dram_tensor("out", [OC, OUTF], f32, kind="ExternalOutput")

    with TileContext(nc) as tc:
        with (
            tc.tile_pool(name="const", bufs=1) as cpool,
            tc.tile_pool(name="psum", bufs=4, space="PSUM") as ppool,
        ):
            xrep = cpool.tile([128, SLABF], mdt)
            # partition p = dl*32+ic holds xs[ic, dl : dl+SLABF] (w-shift by dl)
            for dl in range(4):
                nc.sync.dma_start(
                    out=xrep[dl * IC : (dl + 1) * IC, :], in_=xs[:, dl : dl + SLABF]
                )
            kt_sb = cpool.tile([128, NTAPS_LJ * OC], mdt)
            nc.sync.dma_start(out=kt_sb, in_=kt[:, :])
            kt4_sb = cpool.tile([IC, NTAPS_LJ * OC], mdt)
            nc.sync.dma_start(out=kt4_sb, in_=kt4[:, :])
            bias_sb = cpool.tile([OC, 1], f32)
            nc.sync.dma_start(out=bias_sb, in_=bias[:, :])
            obuf = cpool.tile([OC, OUTF], f32)

            # view xrep free dim as (row, w) where row = d*HP + h
            xrep_r = xrep.rearrange("p (r w) -> p r w", w=WP)

            for t in range(8):  # out tile: 512 outputs = 16 h-rows x 32 w
                d, h0 = divmod(t, 2)
                h0 *= 16
                ps = ppool.tile([OC, 512], f32)
                for lj in range(NTAPS_LJ):
                    l, j = divmod(lj, 5)
                    r = (d + l) * HP + h0 + j
                    rhs = xrep_r[:, r : r + 16, 0:W]
                    nc.tensor.matmul(
                        ps,
                        kt_sb[:, lj * OC : (lj + 1) * OC],
                        rhs,
                        start=(lj == 0),
                        stop=False,
                    )
                    rhs4 = xrep_r[0:IC, r : r + 16, 4 : 4 + W]
                    nc.tensor.matmul(
                        ps,
                        kt4_sb[:, lj * OC : (lj + 1) * OC],
                        rhs4,
                        start=False,
                        stop=(lj == NTAPS_LJ - 1),
                    )
                nc.vector.tensor_scalar_add(
                    out=obuf[:, t * 512 : (t + 1) * 512], in0=ps, scalar1=bias_sb
                )
            nc.sync.dma_start(out=out[:, :], in_=obuf)
    nc.finalize()
    _NC_CACHE[key] = nc
    return nc


def kernel(x, weight, P, bias, mm="bf16", ver="v1"):
    import ml_dtypes

    x = np.ascontiguousarray(np.asarray(x, dtype=np.float32))
    weight = np.asarray(weight, dtype=np.float32)
    P = np.asarray(P, dtype=np.float32)
    bias = np.asarray(bias, dtype=np.float32)
    mnp = {"f32": np.float32, "bf16": ml_dtypes.bfloat16}[mm]

    K = _construct_K(weight, P)  # (oc, ic, l, j, i)
    # lhsT layouts: partition=(i, ic), free=(l*5+j slot, oc)
    Kt = K.transpose(4, 1, 2, 3, 0)  # (i, ic, l, j, oc)
    KtF = Kt.reshape(5, IC, NTAPS_LJ, OC)
    bias_in = np.ascontiguousarray(bias.reshape(OC, 1))

    xpad = np.pad(x, ((0, 0), (0, 0), (PAD, PAD), (PAD, PAD), (PAD, PAD)))

    if ver == "v0":
        kt = np.ascontiguousarray(KtF[:4].reshape(128, NTAPS_LJ * OC).astype(mnp))
        kt4 = np.ascontiguousarray(KtF[4].reshape(IC, NTAPS_LJ * OC).astype(mnp))
        extra = {"kt": kt, "kt4": kt4}
        build = _build_nc
    else:
        kta = np.ascontiguousarray(
            KtF[:4][:, :, LJ_A, :].reshape(128, len(LJ_A) * OC).astype(mnp)
        )
        ktb = np.ascontiguousarray(
            KtF[:4][:, :, LJ_B, :].reshape(128, len(LJ_B) * OC).astype(mnp)
        )
        # ktd: partition (l, ic) for l=0..3, free (j, oc): taps (l, j, i=4)
        ktd = np.zeros((128, 5 * OC), mnp)
        for j in range(5):
            for l in range(4):
                ktd[32 * l : 32 * (l + 1), j * OC : (j + 1) * OC] = KtF[
                    4, :, l * 5 + j, :
                ].astype(mnp)
        # ktj: partition (j, ic) for j=0..3: taps (l=4, j, i=4)
        ktj = np.zeros((128, OC), mnp)
        for j in range(4):
            ktj[32 * j : 32 * (j + 1), :] = KtF[4, :, 4 * 5 + j, :].astype(mnp)
        kt5 = np.ascontiguousarray(KtF[4, :, 24, :].astype(mnp))  # (l=4,j=4,i=4)
        extra = {"kta": kta, "ktb": ktb, "ktd": ktd, "ktj": ktj, "kt5": kt5}
        build = _build_nc_packed

    in_maps = []
    for ci in range(8):
        n, dc = divmod(ci, 4)
        slab = xpad[n, :, 4 * dc : 4 * dc + DSLAB].reshape(IC, SLABF)
        xs = np.zeros((IC, XS_COLS), mnp)
        xs[:, :SLABF] = slab.astype(mnp)
        in_maps.append({"xs": xs, "bias": bias_in, **extra})

    global _last_in_maps, _last_mm, _last_build
    _last_in_maps = in_maps
    _last_mm = mm
    _last_build = build
    nc = build(mm)
    res = run_bass_kernel_spmd(nc, in_maps, core_ids=list(range(8)))

    out = np.empty((N, OC, D, H, W), np.float32)
    for ci in range(8):
        n, dc = divmod(ci, 4)
        out[n, :, 4 * dc : 4 * dc + DCHUNK] = res.results[ci]["out"].reshape(
            OC, DCHUNK, H, W
        )
    return out



# revision 6
# speedup vs baseline: 2.4895x; 2.4895x over previous
"""Dcls3d (learnable-position dilated conv3d) Trainium2 kernel.

Reference computes:
  K = trilinear-scatter(weight, P) -> (64, 32, 5, 5, 5)
  out = conv3d(x, K, stride 1, pad 2) + bias     x: (2,32,16,32,32) -> out: (2,64,16,32,32)

Strategy (8 cores): shard (batch n in {0,1}) x (4 chunks of 4 output d-planes).
Each core runs an implicit-GEMM direct conv:
  - input slab (zero-padded on host) replicated 4x in SBUF, w-shifted by
    delta=0..3, giving a 128-partition (delta, ic) contraction axis.
  - for each of 25 (l, j) kernel-tap pairs: one matmul contracting
    (4 w-taps x 32 ic) = 128, M=64 out-channels, N=512 outputs, accumulating
    in PSUM; the i=4 leftover tap runs as a K=32 matmul off the delta-group.
  - bias added during PSUM->SBUF copyback; one 1MB store per core.
"""

import dataclasses

import numpy as np

import concourse.bass as bass
import concourse.bacc as bacc
import concourse.mybir as mybir
from concourse.bass_utils import run_bass_kernel_spmd
from concourse.tile import TileContext

# ---- problem constants (hardcoded per contract) ----
N, IC, D, H, W = 2, 32, 16, 32, 32
OC = 64
KC = 16
PAD = 2
DP, HP, WP = D + 2 * PAD, H + 2 * PAD, W + 2 * PAD  # 20, 36, 36
DCHUNK = 4              # output d-planes per core
DSLAB = DCHUNK + 4      # input d-planes per core (halo 2 each side)
SLABF = DSLAB * HP * WP  # 8*36*36 = 10368
XS_COLS = SLABF + 4     # slack so the delta-shifted loads stay in bounds
NTAPS_LJ = 25
OUTF = DCHUNK * H * W   # 4096 outputs per (core, oc)

_NC_CACHE = {}


def _construct_K(weight, P):
    """Exact numpy port of reference.construct_kernel for ks=(5,5,5)."""
    Pp = P + np.float32(2.0)
    Pf = np.floor(Pp)
    R = Pp - Pf
    P1, P2, P3 = Pf[0], Pf[1], Pf[2]
    R1, R2, R3 = R[0], R[1], R[2]
    g = np.arange(5, dtype=P.dtype)[:, None, None, None]
    aL = (g == P1) * (1.0 - R1) + (g == P1 + 1.0) * R1
    aJ = (g == P3) * (1.0 - R3) + (g == P3 + 1.0) * R3
    aI = (g == P2) * (1.0 - R2) + (g == P2 + 1.0) * R2
    K = np.einsum("ock,lock,jock,iock->oclji", weight, aL, aJ, aI, optimize=True)
    return np.ascontiguousarray(K.astype(np.float32))


LJ_A = [lj for lj in range(NTAPS_LJ) if lj % 2 == 0]  # col-group 0 taps
LJ_B = [lj for lj in range(NTAPS_LJ) if lj % 2 == 1]  # col-group 1 taps
ROW_PACK = False  # leftover i=4 taps spread across PE row groups


def _build_nc_packed(mm="bf16"):
    """v1: col-group packed (2 taps concurrently on PE) + row-packed i=4."""
    key = ("v1", mm, ROW_PACK)
    if key in _NC_CACHE:
        return _NC_CACHE[key]
    f32 = mybir.dt.float32
    mdt = {"f32": f32, "bf16": mybir.dt.bfloat16}[mm]
    nc = bacc.Bacc()
    xs = nc.dram_tensor("xs", [IC, XS_COLS], mdt, kind="ExternalInput")
    kta = nc.dram_tensor("kta", [128, len(LJ_A) * OC], mdt, kind="ExternalInput")
    ktb = nc.dram_tensor("ktb", [128, len(LJ_B) * OC], mdt, kind="ExternalInput")
    ktd = nc.dram_tensor("ktd", [128, 5 * OC], mdt, kind="ExternalInput")
    ktj = nc.dram_tensor("ktj", [128, OC], mdt, kind="ExternalInput")
    kt5 = nc.dram_tensor("kt5", [IC, OC], mdt, kind="ExternalInput")
    bias = nc.dram_tensor("bias", [OC, 1], f32, kind="ExternalInput")
    out = nc.dram_tensor("out", [OC, OUTF], f32, kind="ExternalOutput")

    HALF = 6 * HP * WP  # six d-planes per xrep half
    with TileContext(nc) as tc:
        with (
            tc.tile_pool(name="const", bufs=1) as cpool,
            tc.tile_pool(name="psum", bufs=8, space="PSUM") as ppool,
        ):
            kta_sb = cpool.tile([128, len(LJ_A) * OC], mdt)
            nc.sync.dma_start(out=kta_sb, in_=kta[:, :])
            ktb_sb = cpool.tile([128, len(LJ_B) * OC], mdt)
            nc.sync.dma_start(out=ktb_sb, in_=ktb[:, :])
            ktd_sb = cpool.tile([128, 5 * OC], mdt)
            nc.sync.dma_start(out=ktd_sb, in_=ktd[:, :])
            ktj_sb = cpool.tile([128, OC], mdt)
            nc.sync.dma_start(out=ktj_sb, in_=ktj[:, :])
            kt5_sb = cpool.tile([IC, OC], mdt)
            nc.sync.dma_start(out=kt5_sb, in_=kt5[:, :])
            bias_sb = cpool.tile([OC, 1], f32)
            nc.sync.dma_start(out=bias_sb, in_=bias[:, :])
            # input slab split in two halves (planes 0-5 / 2-7) so out d=0,1
            # compute starts while the second half still loads
            xrepA = cpool.tile([128, HALF], mdt)
            xrepB = cpool.tile([128, HALF], mdt)
            for dl in range(4):
                nc.sync.dma_start(
                    out=xrepA[dl * IC : (dl + 1) * IC, :], in_=xs[:, dl : dl + HALF]
                )
            for dl in range(4):
                nc.sync.dma_start(
                    out=xrepB[dl * IC : (dl + 1) * IC, :],
                    in_=xs[:, 2 * HP * WP + dl : 2 * HP * WP + dl + HALF],
                )
            obufs = [cpool.tile([OC, H * W], f32, name=f"obuf{d}") for d in range(4)]

            # d-shifted replication for the i=4 taps: partition group
            # lam holds xs shifted by lam d-planes AND +4 in w, so one
            # K=128 matmul covers taps (l=lam, j, i=4) for lam=0..3.
            DWIN = 4 * HP * WP
            xrepD = cpool.tile([128, DWIN], mdt)
            for lam in range(4):
                o = lam * HP * WP + 4
                nc.sync.dma_start(
                    out=xrepD[lam * IC : (lam + 1) * IC, :], in_=xs[:, o : o + DWIN]
                )
            # h-row (j) shifted replication for taps (l=4, j=0..3, i=4):
            # partition group mu holds planes 4..7 shifted by mu rows and +4 w
            JWIN = 5040
            xrepJ = cpool.tile([128, JWIN], mdt)
            for mu in range(4):
                o = 4 * HP * WP + mu * WP + 4
                nc.sync.dma_start(
                    out=xrepJ[mu * IC : (mu + 1) * IC, :], in_=xs[:, o : o + JWIN]
                )

            xrepA_r = xrepA.rearrange("p (r w) -> p r w", w=WP)
            xrepB_r = xrepB.rearrange("p (r w) -> p r w", w=WP)
            xrepD_r = xrepD.rearrange("p (r w) -> p r w", w=WP)
            xrepJ_r = xrepJ.rearrange("p (r w) -> p r w", w=WP)

            def tile_geom(t):
                d, h0 = divmod(t, 2)
                h0 *= 16
                xr = xrepA_r if d < 2 else xrepB_r
                dbase = 0 if d < 2 else 2
                return d, h0, xr, dbase

            # pass 1: all w-packed taps (need only xrepA/xrepB) for all 8
            # tiles -- 8 psum banks accumulate concurrently, so the PE never
            # stalls on the later xrepD/xrepJ DMAs.
            pss = []
            for t in range(8):
                d, h0, xrep_r, dbase = tile_geom(t)
                ps = ppool.tile([128, 512], f32)
                pss.append(ps)
                for s in range(len(LJ_A)):
                    for grp, ljs, ktsb in ((0, LJ_A, kta_sb), (1, LJ_B, ktb_sb)):
                        if s >= len(ljs):
                            continue
                        lj = ljs[s]
                        l, j = divmod(lj, 5)
                        r = (d + l - dbase) * HP + h0 + j
                        nc.tensor.matmul(
                            ps[grp * 64 : grp * 64 + 64, :],
                            ktsb[:, s * OC : (s + 1) * OC],
                            xrep_r[:, r : r + 16, 0:W],
                            start=(s == 0),
                            stop=False,
                            skip_group_check=True,
                            tile_position=(0, grp * 64),
                        )
            # pass 2: i=4 closers off xrepD/xrepJ + corner single + epilogue
            for t in range(8):
                d, h0, xrep_r, dbase = tile_geom(t)
                ps = pss[t]
                for j in range(5):
                    grp = j % 2
                    nc.tensor.matmul(
                        ps[grp * 64 : grp * 64 + 64, :],
                        ktd_sb[:, j * OC : (j + 1) * OC],
                        xrepD_r[:, d * HP + h0 + j : d * HP + h0 + j + 16, 0:W],
                        start=False,
                        stop=False,
                        skip_group_check=True,
                        tile_position=(0, grp * 64),
                    )
                nc.tensor.matmul(
                    ps[64:128, :],
                    ktj_sb[:, :],
                    xrepJ_r[:, d * HP + h0 : d * HP + h0 + 16, 0:W],
                    start=False,
                    stop=True,
                    skip_group_check=True,
                    tile_position=(0, 64),
                )
                r45 = (d + 4 - dbase) * HP + h0 + 4  # tap (l=4, j=4)
                nc.tensor.matmul(
                    ps[0:64, :],
                    kt5_sb[0:IC, :],
                    xrep_r[0:IC, r45 : r45 + 16, 4 : 4 + W],
                    start=False,
                    stop=True,
                    skip_group_check=True,
                    tile_position=(0, 0),
                )
                oslice = obufs[d][:, (t % 2) * 512 : (t % 2) * 512 + 512]
                nc.vector.tensor_scalar_add(out=oslice, in0=ps[0:64, :], scalar1=bias_sb)
                nc.vector.tensor_tensor(
                    out=oslice, in0=ps[64:128, :], in1=oslice,
                    op=mybir.AluOpType.add,
                )
                if t % 2 == 1:
                    nc.sync.dma_start(
                        out=out[:, d * H * W : (d + 1) * H * W], in_=obufs[d]
                    )
    nc.finalize()
    _NC_CACHE[key] = nc
    return nc


def _build_nc(mm="bf16"):
    key = ("v0", mm)
    if key in _NC_CACHE:
        return _NC_CACHE[key]
    f32 = mybir.dt.float32
    mdt = {"f32": f32, "bf16": mybir.dt.bfloat16}[mm]
    nc = bacc.Bacc()
    xs = nc.dram_tensor("xs", [IC, XS_COLS], mdt, kind="ExternalInput")
    kt = nc.dram_tensor("kt", [128, NTAPS_LJ * OC], mdt, kind="ExternalInput")
    kt4 = nc.dram_tensor("kt4", [IC, NTAPS_LJ * OC], mdt, kind="ExternalInput")
    bias = nc.dram_tensor("bias", [OC, 1], f32, kind="ExternalInput")
    out = nc.dram_tensor("out", [OC, OUTF], f32, kind="ExternalOutput")

    with TileContext(nc) as tc:
        with (
            tc.tile_pool(name="const", bufs=1) as cpool,
            tc.tile_pool(name="psum", bufs=4, space="PSUM") as ppool,
        ):
            xrep = cpool.tile([128, SLABF], mdt)
            # partition p = dl*32+ic holds xs[ic, dl : dl+SLABF] (w-shift by dl)
            for dl in range(4):
                nc.sync.dma_start(
                    out=xrep[dl * IC : (dl + 1) * IC, :], in_=xs[:, dl : dl + SLABF]
                )
            kt_sb = cpool.tile([128, NTAPS_LJ * OC], mdt)
            nc.sync.dma_start(out=kt_sb, in_=kt[:, :])
            kt4_sb = cpool.tile([IC, NTAPS_LJ * OC], mdt)
            nc.sync.dma_start(out=kt4_sb, in_=kt4[:, :])
            bias_sb = cpool.tile([OC, 1], f32)
            nc.sync.dma_start(out=bias_sb, in_=bias[:, :])
            obuf = cpool.tile([OC, OUTF], f32)

            # view xrep free dim as (row, w) where row = d*HP + h
            xrep_r = xrep.rearrange("p (r w) -> p r w", w=WP)

            for t in range(8):  # out tile: 512 outputs = 16 h-rows x 32 w
                d, h0 = divmod(t, 2)
                h0 *= 16
                ps = ppool.tile([OC, 512], f32)
                for lj in range(NTAPS_LJ):
                    l, j = divmod(lj, 5)
                    r = (d + l) * HP + h0 + j
                    rhs = xrep_r[:, r : r + 16, 0:W]
                    nc.tensor.matmul(
                        ps,
                        kt_sb[:, lj * OC : (lj + 1) * OC],
                        rhs,
                        start=(lj == 0),
                        stop=False,
                    )
                    rhs4 = xrep_r[0:IC, r : r + 16, 4 : 4 + W]
                    nc.tensor.matmul(
                        ps,
                        kt4_sb[:, lj * OC : (lj + 1) * OC],
                        rhs4,
                        start=False,
                        stop=(lj == NTAPS_LJ - 1),
                    )
                nc.vector.tensor_scalar_add(
                    out=obuf[:, t * 512 : (t + 1) * 512], in0=ps, scalar1=bias_sb
                )
            nc.sync.dma_start(out=out[:, :], in_=obuf)
    nc.finalize()
    _NC_CACHE[key] = nc
    return nc


# ---------------------------------------------------------------------------
# v2: d-paired M=128 bf16 core + fp8 DoubleRow edge/face taps.
#
# Per core: 4 output d-planes (D=0..3), slab = 8 padded planes (S=0..7).
# 4 "banks", each = (pd in {0,2}) x (ht in {0,1}): psum partitions 0-63
# accumulate tile Ta=(D=pd), 64-127 tile Tb=(D=pd+1); both tiles share the
# same 16-row input windows (plane S=pd+p' serves Ta as tap l=p', Tb as
# l=p'-1), so every bf16 matmul runs the full 128-wide PE.
#   bf16 main windows:  p'=1..4, j=0..4  -> taps i=0..3 via 4 w-shifted
#     slab copies packed in K=128 (xrep).
#   fp8 DoubleRow windows (K-tiles pack j=mu+4t): edge-lo p'=0 (Ta l=0,
#     all i), face p'=1..4 (i=4), edge-hi p'=5 (Tb l=4, all i), via 4
#     row-shifted fp8 slab copies (xJ8). fp8 weights are scaled x16 into a
#     separate psum bank; the epilogue rescales by 1/16 and adds bias on
#     the Activation engine, then adds the main psum on DVE.
# A warmup block of tiny matmuls holds the PE busy from t=0 so the cost
# model's p-state ramp finishes before the first real matmul dispatches.
# ---------------------------------------------------------------------------
V2_WARM_N = 64   # free size of each warmup matmul
V2_WARM_W = 52   # number of warmup matmuls

PLANE = HP * WP          # 1296
XREP_COLS = 6 * PLANE    # planes S=1..6
XJ8_COLS = 8 * PLANE     # planes S=0..7
XSB_COLS = PLANE + XREP_COLS + 8       # bf16 slab src: cols 1296+d .. +7776
XS8_COLS = XJ8_COLS + 3 * WP + 8       # fp8 slab src: cols mu*36 .. +10368
FP8_SCALE = 16.0
N_MAIN_BLK = 20          # (p'-1)*5 + j
N_FP8_BLK = 14           # 0-4 edge-lo i, 5-8 face p'=1..4, 9-13 edge-hi i


def _build_nc_v2(mm="bf16"):
    key = ("v2", V2_WARM_N, V2_WARM_W)
    if key in _NC_CACHE:
        return _NC_CACHE[key]
    f32 = mybir.dt.float32
    bf16 = mybir.dt.bfloat16
    fp8 = mybir.dt.float8e4
    nc = bacc.Bacc()
    xsb = nc.dram_tensor("xsb", [IC, XSB_COLS], bf16, kind="ExternalInput")
    xs8 = nc.dram_tensor("xs8", [IC, XS8_COLS], fp8, kind="ExternalInput")
    ktm = nc.dram_tensor("ktm", [128, N_MAIN_BLK * 128], bf16, kind="ExternalInput")
    kt8 = nc.dram_tensor("kt8", [128, N_FP8_BLK * 256], fp8, kind="ExternalInput")
    bias = nc.dram_tensor("bias", [OC, 1], f32, kind="ExternalInput")
    out = nc.dram_tensor("out", [OC, 4 * H * W], f32, kind="ExternalOutput")

    with TileContext(nc) as tc:
        with (
            tc.tile_pool(name="const", bufs=1) as cpool,
            tc.tile_pool(name="psum", bufs=1, space="PSUM") as ppool,
        ):
            wt = cpool.tile([1, V2_WARM_N], bf16)
            bias_sb = cpool.tile([OC, 1], f32)
            ktm_sb = cpool.tile([128, N_MAIN_BLK * 128], bf16)
            kt8_sb = cpool.tile([128, N_FP8_BLK * 256], fp8)
            xrep = cpool.tile([128, XREP_COLS], bf16)
            xJ8 = cpool.tile([128, XJ8_COLS], fp8)
            obufs = [cpool.tile([OC, 2 * 512], f32, name=f"ob{b}") for b in range(4)]
            psM = [ppool.tile([128, 512], f32, name=f"psM{b}") for b in range(4)]
            psF = [ppool.tile([128, 512], f32, name=f"psF{b}") for b in range(4)]

            # -- warmup: PE busy from t~0 on zeroed junk so the p-state
            # ramp completes while input DMAs stream in.
            nc.vector.memset(wt, 0)
            for _ in range(V2_WARM_W):
                nc.tensor.matmul(
                    psF[3][0:1, 0:V2_WARM_N], wt[0:1, 0:1], wt[0:1, :],
                    start=True, stop=True, skip_group_check=True,
                )

            # -- DMA stream (SP queue order == arrival order)
            nc.sync.dma_start(out=bias_sb, in_=bias[:, :])
            nc.sync.dma_start(out=ktm_sb[:, : 10 * 128], in_=ktm[:, : 10 * 128])
            H2 = 2 * PLANE
            for dl in range(4):  # xrep planes S=1..2 (q=0..1)
                nc.sync.dma_start(
                    out=xrep[dl * IC : (dl + 1) * IC, 0:H2],
                    in_=xsb[:, PLANE + dl : PLANE + dl + H2],
                )
            nc.sync.dma_start(out=kt8_sb, in_=kt8[:, :])
            for dl in range(4):  # xrep planes S=3..4 (q=2..3)
                nc.sync.dma_start(
                    out=xrep[dl * IC : (dl + 1) * IC, H2 : 2 * H2],
                    in_=xsb[:, PLANE + dl + H2 : PLANE + dl + 2 * H2],
                )
            H6 = 6 * PLANE
            for mu in range(4):  # xJ8 planes S=0..5
                nc.sync.dma_start(
                    out=xJ8[mu * IC : (mu + 1) * IC, 0:H6],
                    in_=xs8[:, mu * WP : mu * WP + H6],
                )
            nc.sync.dma_start(out=ktm_sb[:, 10 * 128 :], in_=ktm[:, 10 * 128 :])
            for dl in range(4):  # xrep planes S=5..6 (q=4..5)
                nc.sync.dma_start(
                    out=xrep[dl * IC : (dl + 1) * IC, 2 * H2 : XREP_COLS],
                    in_=xsb[:, PLANE + dl + 2 * H2 : PLANE + dl + XREP_COLS],
                )
            for mu in range(4):  # xJ8 planes S=6..7
                nc.sync.dma_start(
                    out=xJ8[mu * IC : (mu + 1) * IC, H6:XJ8_COLS],
                    in_=xs8[:, mu * WP + H6 : mu * WP + XJ8_COLS],
                )

            xrep_r = xrep.rearrange("p (r w) -> p r w", w=WP)
            xj8_pdim = list(xJ8[:, :].ap[0])

            mstate = {}  # (bank, kind) -> started?

            def mm_main(pd, ht, pprime, j, stop):
                b = pd + ht
                lhsT = ktm_sb[:, ((pprime - 1) * 5 + j) * 128 :][:, :128]
                R = (pd + pprime - 1) * HP + ht * 16 + j
                st = (b, "m") not in mstate
                mstate[(b, "m")] = True
                nc.tensor.matmul(
                    psM[b], lhsT, xrep_r[:, R : R + 16, 0:W],
                    start=st, stop=stop, skip_group_check=True,
                )

            def mm_fp8(pd, ht, blk, S_off, i, stop):
                b = pd + ht
                lhsT = kt8_sb[:, blk * 256 : (blk + 1) * 256].rearrange(
                    "p (t m) -> p t m", t=2
                )
                off = (pd + S_off) * PLANE + ht * 16 * WP + i
                base = xJ8[:, off : off + 716]
                rhs = dataclasses.replace(
                    base, ap=[xj8_pdim, [4 * WP, 2], [WP, 16], [1, W]]
                )
                st = (b, "f") not in mstate
                mstate[(b, "f")] = True
                nc.tensor.matmul(
                    psF[b], lhsT, rhs,
                    start=st, stop=stop,
                    perf_mode=mybir.MatmulPerfMode.DoubleRow,
                    skip_group_check=True,
                )

            def epilogue(pd, ht):
                b = pd + ht
                ob = obufs[b]
                for t in range(2):
                    nc.scalar.activation(
                        out=ob[:, t * 512 : (t + 1) * 512],
                        in_=psF[b][t * 64 : (t + 1) * 64, :],
                        func=mybir.ActivationFunctionType.Identity,
                        bias=bias_sb, scale=1.0 / FP8_SCALE,
                    )
                for t in range(2):
                    nc.vector.tensor_tensor(
                        out=ob[:, t * 512 : (t + 1) * 512],
                        in0=ob[:, t * 512 : (t + 1) * 512],
                        in1=psM[b][t * 64 : (t + 1) * 64, :],
                        op=mybir.AluOpType.add,
                    )
                oview = out[:, :].rearrange("o (d t x) -> o d t x", d=4, t=2)
                nc.sync.dma_start(
                    out=oview[:, pd : pd + 2, ht : ht + 1, :],
                    in_=ob.rearrange("o (t x) -> o t x", t=2),
                )

            # Ph1: pd=0 main p'=1,2 (planes S=1..2)
            for ht in range(2):
                for pprime in (1, 2):
                    for j in range(5):
                        mm_main(0, ht, pprime, j, stop=False)
            # Ph2: pd=0 main p'=3,4 + pd=2 main p'=1,2 (S=3..4)
            for ht in range(2):
                for pprime in (3, 4):
                    for j in range(5):
                        mm_main(0, ht, pprime, j, stop=(pprime == 4 and j == 4))
            for ht in range(2):
                for pprime in (1, 2):
                    for j in range(5):
                        mm_main(2, ht, pprime, j, stop=False)
            # Ph3: pd=0 all fp8 (xJ8 planes 0..5), retire pd=0 banks
            for ht in range(2):
                for i in range(5):
                    mm_fp8(0, ht, i, 0, i, stop=False)
                for pprime in range(1, 5):
                    mm_fp8(0, ht, 4 + pprime, pprime, 4, stop=False)
                for i in range(5):
                    mm_fp8(0, ht, 9 + i, 5, i, stop=(i == 4))
                epilogue(0, ht)
            # Ph4: pd=2 main p'=3,4 (S=5..6)
            for ht in range(2):
                for pprime in (3, 4):
                    for j in range(5):
                        mm_main(2, ht, pprime, j, stop=(pprime == 4 and j == 4))
            # Ph5: pd=2 fp8 (xJ8 planes 2..7), retire pd=2 banks
            for ht in range(2):
                for i in range(5):
                    mm_fp8(2, ht, i, 0, i, stop=False)
                for pprime in range(1, 5):
                    mm_fp8(2, ht, 4 + pprime, pprime, 4, stop=False)
                for i in range(5):
                    mm_fp8(2, ht, 9 + i, 5, i, stop=(i == 4))
                epilogue(2, ht)
    nc.finalize()
    _NC_CACHE[key] = nc
    return nc


def _prep_v2_weights(K, mnp, f8np):
    """ktm [128, 20*128] bf16; kt8 [128, 14*256] fp8 (x16)."""
    ktm = np.zeros((128, N_MAIN_BLK * 128), np.float32)
    for pprime in range(1, 5):
        for j in range(5):
            blk = (pprime - 1) * 5 + j
            for dl in range(4):
                r = slice(dl * IC, (dl + 1) * IC)
                # cols m: Ta tap (l=p', j, i=dl); cols 64+m: Tb (p'-1, j, dl)
                ktm[r, blk * 128 : blk * 128 + 64] = K[:, :, pprime, j, dl].T
                ktm[r, blk * 128 + 64 : (blk + 1) * 128] = K[
                    :, :, pprime - 1, j, dl
                ].T
    kt8 = np.zeros((128, N_FP8_BLK, 2, 128), np.float32)
    for mu in range(4):
        r = slice(mu * IC, (mu + 1) * IC)
        for t in range(2):
            j = mu + 4 * t
            if j > 4:
                continue
            for i in range(5):
                kt8[r, i, t, 0:64] = FP8_SCALE * K[:, :, 0, j, i].T  # edge-lo Ta
                kt8[r, 9 + i, t, 64:128] = FP8_SCALE * K[:, :, 4, j, i].T  # hi Tb
            for pprime in range(1, 5):
                kt8[r, 4 + pprime, t, 0:64] = FP8_SCALE * K[:, :, pprime, j, 4].T
                kt8[r, 4 + pprime, t, 64:128] = (
                    FP8_SCALE * K[:, :, pprime - 1, j, 4].T
                )
    return (
        np.ascontiguousarray(ktm.astype(mnp)),
        np.ascontiguousarray(kt8.reshape(128, N_FP8_BLK * 256).astype(f8np)),
    )


def _kernel_v2(x, weight, P, bias):
    import ml_dtypes

    mnp = ml_dtypes.bfloat16
    f8np = ml_dtypes.float8_e4m3
    K = _construct_K(weight, P)
    ktm_np, kt8_np = _prep_v2_weights(K, mnp, f8np)
    bias_in = np.ascontiguousarray(bias.reshape(OC, 1))

    xpad = np.pad(x, ((0, 0), (0, 0), (PAD, PAD), (PAD, PAD), (PAD, PAD)))
    in_maps = []
    for ci in range(8):
        n, dc = divmod(ci, 4)
        slab = xpad[n, :, 4 * dc : 4 * dc + DSLAB].reshape(IC, SLABF)
        xsb = np.zeros((IC, XSB_COLS), mnp)
        xsb[:, : min(SLABF, XSB_COLS)] = slab[:, :XSB_COLS].astype(mnp)
        xs8 = np.zeros((IC, XS8_COLS), f8np)
        xs8[:, :SLABF] = slab.astype(f8np)
        in_maps.append(
            {"xsb": xsb, "xs8": xs8, "ktm": ktm_np, "kt8": kt8_np, "bias": bias_in}
        )

    global _last_in_maps, _last_mm, _last_build
    _last_in_maps = in_maps
    _last_mm = "bf16"
    _last_build = _build_nc_v2
    nc = _build_nc_v2()
    res = run_bass_kernel_spmd(nc, in_maps, core_ids=list(range(8)))

    out = np.empty((N, OC, D, H, W), np.float32)
    for ci in range(8):
        n, dc = divmod(ci, 4)
        out[n, :, 4 * dc : 4 * dc + DCHUNK] = res.results[ci]["out"].reshape(
            OC, DCHUNK, H, W
        )
    return out


def kernel(x, weight, P, bias, mm="bf16", ver="v2"):
    import ml_dtypes

    x = np.ascontiguousarray(np.asarray(x, dtype=np.float32))
    weight = np.asarray(weight, dtype=np.float32)
    P = np.asarray(P, dtype=np.float32)
    bias = np.asarray(bias, dtype=np.float32)
    if ver == "v2":
        return _kernel_v2(x, weight, P, bias)
    mnp = {"f32": np.float32, "bf16": ml_dtypes.bfloat16}[mm]

    K = _construct_K(weight, P)  # (oc, ic, l, j, i)
    # lhsT layouts: partition=(i, ic), free=(l*5+j slot, oc)
    Kt = K.transpose(4, 1, 2, 3, 0)  # (i, ic, l, j, oc)
    KtF = Kt.reshape(5, IC, NTAPS_LJ, OC)
    bias_in = np.ascontiguousarray(bias.reshape(OC, 1))

    xpad = np.pad(x, ((0, 0), (0, 0), (PAD, PAD), (PAD, PAD), (PAD, PAD)))

    if ver == "v0":
        kt = np.ascontiguousarray(KtF[:4].reshape(128, NTAPS_LJ * OC).astype(mnp))
        kt4 = np.ascontiguousarray(KtF[4].reshape(IC, NTAPS_LJ * OC).astype(mnp))
        extra = {"kt": kt, "kt4": kt4}
        build = _build_nc
    else:
        kta = np.ascontiguousarray(
            KtF[:4][:, :, LJ_A, :].reshape(128, len(LJ_A) * OC).astype(mnp)
        )
        ktb = np.ascontiguousarray(
            KtF[:4][:, :, LJ_B, :].reshape(128, len(LJ_B) * OC).astype(mnp)
        )
        # ktd: partition (l, ic) for l=0..3, free (j, oc): taps (l, j, i=4)
        ktd = np.zeros((128, 5 * OC), mnp)
        for j in range(5):
            for l in range(4):
                ktd[32 * l : 32 * (l + 1), j * OC : (j + 1) * OC] = KtF[
                    4, :, l * 5 + j, :
                ].astype(mnp)
        # ktj: partition (j, ic) for j=0..3: taps (l=4, j, i=4)
        ktj = np.zeros((128, OC), mnp)
        for j in range(4):
            ktj[32 * j : 32 * (j + 1), :] = KtF[4, :, 4 * 5 + j, :].astype(mnp)
        kt5 = np.ascontiguousarray(KtF[4, :, 24, :].astype(mnp))  # (l=4,j=4,i=4)
        extra = {"kta": kta, "ktb": ktb, "ktd": ktd, "ktj": ktj, "kt5": kt5}
        build = _build_nc_packed

    in_maps = []
    for ci in range(8):
        n, dc = divmod(ci, 4)
        slab = xpad[n, :, 4 * dc : 4 * dc + DSLAB].reshape(IC, SLABF)
        xs = np.zeros((IC, XS_COLS), mnp)
        xs[:, :SLABF] = slab.astype(mnp)
        in_maps.append({"xs": xs, "bias": bias_in, **extra})

    global _last_in_maps, _last_mm, _last_build
    _last_in_maps = in_maps
    _last_mm = mm
    _last_build = build
    nc = build(mm)
    res = run_bass_kernel_spmd(nc, in_maps, core_ids=list(range(8)))

    out = np.empty((N, OC, D, H, W), np.float32)
    for ci in range(8):
        n, dc = divmod(ci, 4)
        out[n, :, 4 * dc : 4 * dc + DCHUNK] = res.results[ci]["out"].reshape(
            OC, DCHUNK, H, W
        )
    return out



# revision 11
# speedup vs baseline: 2.7187x; 1.0921x over previous
"""Dcls3d (learnable-position dilated conv3d) Trainium2 kernel.

Reference computes:
  K = trilinear-scatter(weight, P) -> (64, 32, 5, 5, 5)
  out = conv3d(x, K, stride 1, pad 2) + bias     x: (2,32,16,32,32) -> out: (2,64,16,32,32)

Strategy (8 cores): shard (batch n in {0,1}) x (4 chunks of 4 output d-planes).
Each core runs an implicit-GEMM direct conv:
  - input slab (zero-padded on host) replicated 4x in SBUF, w-shifted by
    delta=0..3, giving a 128-partition (delta, ic) contraction axis.
  - for each of 25 (l, j) kernel-tap pairs: one matmul contracting
    (4 w-taps x 32 ic) = 128, M=64 out-channels, N=512 outputs, accumulating
    in PSUM; the i=4 leftover tap runs as a K=32 matmul off the delta-group.
  - bias added during PSUM->SBUF copyback; one 1MB store per core.
"""

import dataclasses

import numpy as np

import concourse.bass as bass
import concourse.bacc as bacc
import concourse.mybir as mybir
from concourse.bass_utils import run_bass_kernel_spmd
from concourse.tile import TileContext

# ---- problem constants (hardcoded per contract) ----
N, IC, D, H, W = 2, 32, 16, 32, 32
OC = 64
KC = 16
PAD = 2
DP, HP, WP = D + 2 * PAD, H + 2 * PAD, W + 2 * PAD  # 20, 36, 36
DCHUNK = 4              # output d-planes per core
DSLAB = DCHUNK + 4      # input d-planes per core (halo 2 each side)
SLABF = DSLAB * HP * WP  # 8*36*36 = 10368
XS_COLS = SLABF + 4     # slack so the delta-shifted loads stay in bounds
NTAPS_LJ = 25
OUTF = DCHUNK * H * W   # 4096 outputs per (core, oc)

_NC_CACHE = {}


def _construct_K(weight, P):
    """Exact numpy port of reference.construct_kernel for ks=(5,5,5)."""
    Pp = P + np.float32(2.0)
    Pf = np.floor(Pp)
    R = Pp - Pf
    P1, P2, P3 = Pf[0], Pf[1], Pf[2]
    R1, R2, R3 = R[0], R[1], R[2]
    g = np.arange(5, dtype=P.dtype)[:, None, None, None]
    aL = (g == P1) * (1.0 - R1) + (g == P1 + 1.0) * R1
    aJ = (g == P3) * (1.0 - R3) + (g == P3 + 1.0) * R3
    aI = (g == P2) * (1.0 - R2) + (g == P2 + 1.0) * R2
    K = np.einsum("ock,lock,jock,iock->oclji", weight, aL, aJ, aI, optimize=True)
    return np.ascontiguousarray(K.astype(np.float32))


LJ_A = [lj for lj in range(NTAPS_LJ) if lj % 2 == 0]  # col-group 0 taps
LJ_B = [lj for lj in range(NTAPS_LJ) if lj % 2 == 1]  # col-group 1 taps
ROW_PACK = False  # leftover i=4 taps spread across PE row groups


def _build_nc_packed(mm="bf16"):
    """v1: col-group packed (2 taps concurrently on PE) + row-packed i=4."""
    key = ("v1", mm, ROW_PACK)
    if key in _NC_CACHE:
        return _NC_CACHE[key]
    f32 = mybir.dt.float32
    mdt = {"f32": f32, "bf16": mybir.dt.bfloat16}[mm]
    nc = bacc.Bacc()
    xs = nc.dram_tensor("xs", [IC, XS_COLS], mdt, kind="ExternalInput")
    kta = nc.dram_tensor("kta", [128, len(LJ_A) * OC], mdt, kind="ExternalInput")
    ktb = nc.dram_tensor("ktb", [128, len(LJ_B) * OC], mdt, kind="ExternalInput")
    ktd = nc.dram_tensor("ktd", [128, 5 * OC], mdt, kind="ExternalInput")
    ktj = nc.dram_tensor("ktj", [128, OC], mdt, kind="ExternalInput")
    kt5 = nc.dram_tensor("kt5", [IC, OC], mdt, kind="ExternalInput")
    bias = nc.dram_tensor("bias", [OC, 1], f32, kind="ExternalInput")
    out = nc.dram_tensor("out", [OC, OUTF], f32, kind="ExternalOutput")

    HALF = 6 * HP * WP  # six d-planes per xrep half
    with TileContext(nc) as tc:
        with (
            tc.tile_pool(name="const", bufs=1) as cpool,
            tc.tile_pool(name="psum", bufs=8, space="PSUM") as ppool,
        ):
            kta_sb = cpool.tile([128, len(LJ_A) * OC], mdt)
            nc.sync.dma_start(out=kta_sb, in_=kta[:, :])
            ktb_sb = cpool.tile([128, len(LJ_B) * OC], mdt)
            nc.sync.dma_start(out=ktb_sb, in_=ktb[:, :])
            ktd_sb = cpool.tile([128, 5 * OC], mdt)
            nc.sync.dma_start(out=ktd_sb, in_=ktd[:, :])
            ktj_sb = cpool.tile([128, OC], mdt)
            nc.sync.dma_start(out=ktj_sb, in_=ktj[:, :])
            kt5_sb = cpool.tile([IC, OC], mdt)
            nc.sync.dma_start(out=kt5_sb, in_=kt5[:, :])
            bias_sb = cpool.tile([OC, 1], f32)
            nc.sync.dma_start(out=bias_sb, in_=bias[:, :])
            # input slab split in two halves (planes 0-5 / 2-7) so out d=0,1
            # compute starts while the second half still loads
            xrepA = cpool.tile([128, HALF], mdt)
            xrepB = cpool.tile([128, HALF], mdt)
            for dl in range(4):
                nc.sync.dma_start(
                    out=xrepA[dl * IC : (dl + 1) * IC, :], in_=xs[:, dl : dl + HALF]
                )
            for dl in range(4):
                nc.sync.dma_start(
                    out=xrepB[dl * IC : (dl + 1) * IC, :],
                    in_=xs[:, 2 * HP * WP + dl : 2 * HP * WP + dl + HALF],
                )
            obufs = [cpool.tile([OC, H * W], f32, name=f"obuf{d}") for d in range(4)]

            # d-shifted replication for the i=4 taps: partition group
            # lam holds xs shifted by lam d-planes AND +4 in w, so one
            # K=128 matmul covers taps (l=lam, j, i=4) for lam=0..3.
            DWIN = 4 * HP * WP
            xrepD = cpool.tile([128, DWIN], mdt)
            for lam in range(4):
                o = lam * HP * WP + 4
                nc.sync.dma_start(
                    out=xrepD[lam * IC : (lam + 1) * IC, :], in_=xs[:, o : o + DWIN]
                )
            # h-row (j) shifted replication for taps (l=4, j=0..3, i=4):
            # partition group mu holds planes 4..7 shifted by mu rows and +4 w
            JWIN = 5040
            xrepJ = cpool.tile([128, JWIN], mdt)
            for mu in range(4):
                o = 4 * HP * WP + mu * WP + 4
                nc.sync.dma_start(
                    out=xrepJ[mu * IC : (mu + 1) * IC, :], in_=xs[:, o : o + JWIN]
                )

            xrepA_r = xrepA.rearrange("p (r w) -> p r w", w=WP)
            xrepB_r = xrepB.rearrange("p (r w) -> p r w", w=WP)
            xrepD_r = xrepD.rearrange("p (r w) -> p r w", w=WP)
            xrepJ_r = xrepJ.rearrange("p (r w) -> p r w", w=WP)

            def tile_geom(t):
                d, h0 = divmod(t, 2)
                h0 *= 16
                xr = xrepA_r if d < 2 else xrepB_r
                dbase = 0 if d < 2 else 2
                return d, h0, xr, dbase

            # pass 1: all w-packed taps (need only xrepA/xrepB) for all 8
            # tiles -- 8 psum banks accumulate concurrently, so the PE never
            # stalls on the later xrepD/xrepJ DMAs.
            pss = []
            for t in range(8):
                d, h0, xrep_r, dbase = tile_geom(t)
                ps = ppool.tile([128, 512], f32)
                pss.append(ps)
                for s in range(len(LJ_A)):
                    for grp, ljs, ktsb in ((0, LJ_A, kta_sb), (1, LJ_B, ktb_sb)):
                        if s >= len(ljs):
                            continue
                        lj = ljs[s]
                        l, j = divmod(lj, 5)
                        r = (d + l - dbase) * HP + h0 + j
                        nc.tensor.matmul(
                            ps[grp * 64 : grp * 64 + 64, :],
                            ktsb[:, s * OC : (s + 1) * OC],
                            xrep_r[:, r : r + 16, 0:W],
                            start=(s == 0),
                            stop=False,
                            skip_group_check=True,
                            tile_position=(0, grp * 64),
                        )
            # pass 2: i=4 closers off xrepD/xrepJ + corner single + epilogue
            for t in range(8):
                d, h0, xrep_r, dbase = tile_geom(t)
                ps = pss[t]
                for j in range(5):
                    grp = j % 2
                    nc.tensor.matmul(
                        ps[grp * 64 : grp * 64 + 64, :],
                        ktd_sb[:, j * OC : (j + 1) * OC],
                        xrepD_r[:, d * HP + h0 + j : d * HP + h0 + j + 16, 0:W],
                        start=False,
                        stop=False,
                        skip_group_check=True,
                        tile_position=(0, grp * 64),
                    )
                nc.tensor.matmul(
                    ps[64:128, :],
                    ktj_sb[:, :],
                    xrepJ_r[:, d * HP + h0 : d * HP + h0 + 16, 0:W],
                    start=False,
                    stop=True,
                    skip_group_check=True,
                    tile_position=(0, 64),
                )
                r45 = (d + 4 - dbase) * HP + h0 + 4  # tap (l=4, j=4)
                nc.tensor.matmul(
                    ps[0:64, :],
                    kt5_sb[0:IC, :],
                    xrep_r[0:IC, r45 : r45 + 16, 4 : 4 + W],
                    start=False,
                    stop=True,
                    skip_group_check=True,
                    tile_position=(0, 0),
                )
                oslice = obufs[d][:, (t % 2) * 512 : (t % 2) * 512 + 512]
                nc.vector.tensor_scalar_add(out=oslice, in0=ps[0:64, :], scalar1=bias_sb)
                nc.vector.tensor_tensor(
                    out=oslice, in0=ps[64:128, :], in1=oslice,
                    op=mybir.AluOpType.add,
                )
                if t % 2 == 1:
                    nc.sync.dma_start(
                        out=out[:, d * H * W : (d + 1) * H * W], in_=obufs[d]
                    )
    nc.finalize()
    _NC_CACHE[key] = nc
    return nc


def _build_nc(mm="bf16"):
    key = ("v0", mm)
    if key in _NC_CACHE:
        return _NC_CACHE[key]
    f32 = mybir.dt.float32
    mdt = {"f32": f32, "bf16": mybir.dt.bfloat16}[mm]
    nc = bacc.Bacc()
    xs = nc.dram_tensor("xs", [IC, XS_COLS], mdt, kind="ExternalInput")
    kt = nc.dram_tensor("kt", [128, NTAPS_LJ * OC], mdt, kind="ExternalInput")
    kt4 = nc.dram_tensor("kt4", [IC, NTAPS_LJ * OC], mdt, kind="ExternalInput")
    bias = nc.dram_tensor("bias", [OC, 1], f32, kind="ExternalInput")
    out = nc.dram_tensor("out", [OC, OUTF], f32, kind="ExternalOutput")

    with TileContext(nc) as tc:
        with (
            tc.tile_pool(name="const", bufs=1) as cpool,
            tc.tile_pool(name="psum", bufs=4, space="PSUM") as ppool,
        ):
            xrep = cpool.tile([128, SLABF], mdt)
            # partition p = dl*32+ic holds xs[ic, dl : dl+SLABF] (w-shift by dl)
            for dl in range(4):
                nc.sync.dma_start(
                    out=xrep[dl * IC : (dl + 1) * IC, :], in_=xs[:, dl : dl + SLABF]
                )
            kt_sb = cpool.tile([128, NTAPS_LJ * OC], mdt)
            nc.sync.dma_start(out=kt_sb, in_=kt[:, :])
            kt4_sb = cpool.tile([IC, NTAPS_LJ * OC], mdt)
            nc.sync.dma_start(out=kt4_sb, in_=kt4[:, :])
            bias_sb = cpool.tile([OC, 1], f32)
            nc.sync.dma_start(out=bias_sb, in_=bias[:, :])
            obuf = cpool.tile([OC, OUTF], f32)

            # view xrep free dim as (row, w) where row = d*HP + h
            xrep_r = xrep.rearrange("p (r w) -> p r w", w=WP)

            for t in range(8):  # out tile: 512 outputs = 16 h-rows x 32 w
                d, h0 = divmod(t, 2)
                h0 *= 16
                ps = ppool.tile([OC, 512], f32)
                for lj in range(NTAPS_LJ):
                    l, j = divmod(lj, 5)
                    r = (d + l) * HP + h0 + j
                    rhs = xrep_r[:, r : r + 16, 0:W]
                    nc.tensor.matmul(
                        ps,
                        kt_sb[:, lj * OC : (lj + 1) * OC],
                        rhs,
                        start=(lj == 0),
                        stop=False,
                    )
                    rhs4 = xrep_r[0:IC, r : r + 16, 4 : 4 + W]
                    nc.tensor.matmul(
                        ps,
                        kt4_sb[:, lj * OC : (lj + 1) * OC],
                        rhs4,
                        start=False,
                        stop=(lj == NTAPS_LJ - 1),
                    )
                nc.vector.tensor_scalar_add(
                    out=obuf[:, t * 512 : (t + 1) * 512], in0=ps, scalar1=bias_sb
                )
            nc.sync.dma_start(out=out[:, :], in_=obuf)
    nc.finalize()
    _NC_CACHE[key] = nc
    return nc


# ---------------------------------------------------------------------------
# v2: d-paired M=128 bf16 core + fp8 DoubleRow edge/face taps.
#
# Per core: 4 output d-planes (D=0..3), slab = 8 padded planes (S=0..7).
# 4 "banks", each = (pd in {0,2}) x (ht in {0,1}): psum partitions 0-63
# accumulate tile Ta=(D=pd), 64-127 tile Tb=(D=pd+1); both tiles share the
# same 16-row input windows (plane S=pd+p' serves Ta as tap l=p', Tb as
# l=p'-1), so every bf16 matmul runs the full 128-wide PE.
#   bf16 main windows:  p'=1..4, j=0..4  -> taps i=0..3 via 4 w-shifted
#     slab copies packed in K=128 (xrep).
#   fp8 DoubleRow windows (K-tiles pack j=mu+4t): edge-lo p'=0 (Ta l=0,
#     all i), face p'=1..4 (i=4), edge-hi p'=5 (Tb l=4, all i), via 4
#     row-shifted fp8 slab copies (xJ8). fp8 weights are scaled x16 into a
#     separate psum bank; the epilogue rescales by 1/16 and adds bias on
#     the Activation engine, then adds the main psum on DVE.
# A warmup block of tiny matmuls holds the PE busy from t=0 so the cost
# model's p-state ramp finishes before the first real matmul dispatches.
# ---------------------------------------------------------------------------
V2_WARM_N = 64   # free size of each warmup matmul
V2_WARM_W = 66   # number of warmup matmuls

PLANE = HP * WP          # 1296
XREP_COLS = 6 * PLANE    # planes S=1..6
XJ8_COLS = 8 * PLANE     # planes S=0..7
XSB_COLS = PLANE + XREP_COLS + 8       # bf16 slab src: cols 1296+d .. +7776
XS8_COLS = XJ8_COLS + 3 * WP + 8       # fp8 slab src: cols mu*36 .. +10368
FP8_SCALE = 16.0
N_MAIN_BLK = 20          # (p'-1)*5 + j
N_FP8_BLK = 14           # 0-4 edge-lo i, 5-8 face p'=1..4, 9-13 edge-hi i


def _build_nc_v2(mm="bf16"):
    key = ("v2", V2_WARM_N, V2_WARM_W)
    if key in _NC_CACHE:
        return _NC_CACHE[key]
    f32 = mybir.dt.float32
    bf16 = mybir.dt.bfloat16
    fp8 = mybir.dt.float8e4
    nc = bacc.Bacc()
    xsb = nc.dram_tensor("xsb", [IC, XSB_COLS], bf16, kind="ExternalInput")
    xs8 = nc.dram_tensor("xs8", [IC, XS8_COLS], fp8, kind="ExternalInput")
    ktm = nc.dram_tensor("ktm", [128, N_MAIN_BLK * 128], bf16, kind="ExternalInput")
    kt8 = nc.dram_tensor("kt8", [128, N_FP8_BLK * 256], fp8, kind="ExternalInput")
    bias = nc.dram_tensor("bias", [OC, 1], f32, kind="ExternalInput")
    out = nc.dram_tensor("out", [OC, 4 * H * W], f32, kind="ExternalOutput")

    with TileContext(nc) as tc:
        with (
            tc.tile_pool(name="const", bufs=1) as cpool,
            tc.tile_pool(name="psum", bufs=1, space="PSUM") as ppool,
        ):
            wt = cpool.tile([1, V2_WARM_N], bf16)
            bias_sb = cpool.tile([OC, 1], f32)
            ktm_sb = cpool.tile([128, N_MAIN_BLK * 128], bf16)
            kt8_sb = cpool.tile([128, N_FP8_BLK * 256], fp8)
            xrep = cpool.tile([128, XREP_COLS], bf16)
            xJ8 = cpool.tile([128, XJ8_COLS], fp8)
            obufs = [cpool.tile([OC, 2 * 512], f32, name=f"ob{b}") for b in range(4)]
            psM = [ppool.tile([128, 512], f32, name=f"psM{b}") for b in range(4)]

            # -- warmup: PE busy from t~0 on zeroed junk so the cost model's
            # p-state ramp completes while the first input DMAs stream in.
            nc.vector.memset(wt, 0)
            for _ in range(V2_WARM_W):
                nc.tensor.matmul(
                    psM[3][0:1, 0:V2_WARM_N], wt[0:1, 0:1], wt[0:1, :],
                    start=True, stop=True, skip_group_check=True,
                )

            # -- DMA stream (SP queue order == arrival order). Each replica
            # load brings all 4 shifted copies in one DMA via an overlapping
            # dram-side access pattern (dim order: shift, ic, cols).
            def load_xrep(c0, c1):
                src = dataclasses.replace(
                    xsb[:, 0 : c1 - c0],
                    ap=[[1, 4], [XSB_COLS, IC], [1, c1 - c0]],
                    offset=PLANE + c0,
                )
                nc.sync.dma_start(out=xrep[:, c0:c1], in_=src)

            def load_xj8(c0, c1):
                src = dataclasses.replace(
                    xs8[:, 0 : c1 - c0],
                    ap=[[WP, 4], [XS8_COLS, IC], [1, c1 - c0]],
                    offset=c0,
                )
                nc.sync.dma_start(out=xJ8[:, c0:c1], in_=src)

            nc.sync.dma_start(out=ktm_sb[:, : 5 * 128], in_=ktm[:, : 5 * 128])
            load_xrep(0, PLANE)                  # q0 (S=1)
            load_xrep(PLANE, 2 * PLANE)          # q1 (S=2)
            nc.sync.dma_start(out=ktm_sb[:, 5 * 128 :], in_=ktm[:, 5 * 128 :])
            load_xrep(2 * PLANE, 4 * PLANE)      # q2-3 (S=3..4)
            nc.sync.dma_start(out=kt8_sb, in_=kt8[:, :])
            nc.sync.dma_start(out=bias_sb, in_=bias[:, :])
            load_xj8(0, 6 * PLANE)               # S=0..5
            load_xrep(4 * PLANE, XREP_COLS)      # q4-5 (S=5..6)
            load_xj8(6 * PLANE, XJ8_COLS)        # S=6..7

            xrep_r = xrep.rearrange("p (r w) -> p r w", w=WP)
            xj8_pdim = list(xJ8[:, :].ap[0])

            mstate = {}

            def mm_main(pd, ht, pprime, j):
                b = pd + ht
                lhsT = ktm_sb[:, ((pprime - 1) * 5 + j) * 128 :][:, :128]
                R = (pd + pprime - 1) * HP + ht * 16 + j
                st = b not in mstate
                mstate[b] = True
                nc.tensor.matmul(
                    psM[b], lhsT, xrep_r[:, R : R + 16, 0:W],
                    start=st, stop=False, skip_group_check=True,
                )

            def mm_fp8(pd, ht, blk, S_off, i, stop=False):
                b = pd + ht
                lhsT = kt8_sb[:, blk * 256 : (blk + 1) * 256].rearrange(
                    "p (t m) -> p t m", t=2
                )
                off = (pd + S_off) * PLANE + ht * 16 * WP + i
                base = xJ8[:, off : off + 716]
                rhs = dataclasses.replace(
                    base, ap=[xj8_pdim, [4 * WP, 2], [WP, 16], [1, W]]
                )
                nc.tensor.matmul(
                    psM[b], lhsT, rhs,
                    start=False, stop=stop,
                    perf_mode=mybir.MatmulPerfMode.DoubleRow,
                    skip_group_check=True,
                )

            def fp8_block(pd, ht):
                for i in range(5):
                    mm_fp8(pd, ht, i, 0, i)
                for pprime in range(1, 5):
                    mm_fp8(pd, ht, 4 + pprime, pprime, 4)
                for i in range(5):
                    mm_fp8(pd, ht, 9 + i, 5, i, stop=(i == 4))

            def epilogue(pd, ht):
                b = pd + ht
                ob = obufs[b]
                # Ta half on Activation, Tb half on Pool — they run in
                # parallel, halving the per-bank epilogue latency.
                nc.scalar.activation(
                    out=ob[:, 0:512],
                    in_=psM[b][0:64, :],
                    func=mybir.ActivationFunctionType.Identity,
                    bias=bias_sb, scale=1.0 / FP8_SCALE,
                )
                nc.vector.tensor_scalar(
                    out=ob[:, 512:1024],
                    in0=psM[b][64:128, :],
                    scalar1=1.0 / FP8_SCALE,
                    scalar2=bias_sb,
                    op0=mybir.AluOpType.mult,
                    op1=mybir.AluOpType.add,
                )
                oview = out[:, :].rearrange("o (d t x) -> o d t x", d=4, t=2)
                nc.sync.dma_start(
                    out=oview[:, pd : pd + 2, ht : ht + 1, :],
                    in_=ob.rearrange("o (t x) -> o t x", t=2),
                )

            # Ph1: pd=0 main p'=1 (q0) then p'=2 (q1)
            for pprime in (1, 2):
                for ht in range(2):
                    for j in range(5):
                        mm_main(0, ht, pprime, j)
            # Ph2: pd=0 main p'=3,4 + pd=2 main p'=1,2 (q2-3)
            for ht in range(2):
                for pprime in (3, 4):
                    for j in range(5):
                        mm_main(0, ht, pprime, j)
            for ht in range(2):
                for pprime in (1, 2):
                    for j in range(5):
                        mm_main(2, ht, pprime, j)
            # Ph3: pd=0 fp8 (xJ8 S=0..5), retire pd=0 banks
            for ht in range(2):
                fp8_block(0, ht)
                epilogue(0, ht)
            # Ph4: pd=2 main p'=3,4 (q4-5)
            for ht in range(2):
                for pprime in (3, 4):
                    for j in range(5):
                        mm_main(2, ht, pprime, j)
            # Ph5: pd=2 fp8 (xJ8 S=2..7), retire pd=2 banks
            for ht in range(2):
                fp8_block(2, ht)
                epilogue(2, ht)
    nc.finalize()
    _NC_CACHE[key] = nc
    return nc


def _prep_v2_weights(K, mnp, f8np):
    """ktm [128, 20*128] bf16; kt8 [128, 14*256] fp8 (x16)."""
    ktm = np.zeros((128, N_MAIN_BLK * 128), np.float32)
    for pprime in range(1, 5):
        for j in range(5):
            blk = (pprime - 1) * 5 + j
            for dl in range(4):
                r = slice(dl * IC, (dl + 1) * IC)
                # cols m: Ta tap (l=p', j, i=dl); cols 64+m: Tb (p'-1, j, dl)
                # x16: all windows accumulate into one psum bank at the fp8
                # weight scale; the epilogue rescales by 1/16 (exact in bf16).
                ktm[r, blk * 128 : blk * 128 + 64] = FP8_SCALE * K[:, :, pprime, j, dl].T
                ktm[r, blk * 128 + 64 : (blk + 1) * 128] = (
                    FP8_SCALE * K[:, :, pprime - 1, j, dl].T
                )
    kt8 = np.zeros((128, N_FP8_BLK, 2, 128), np.float32)
    for mu in range(4):
        r = slice(mu * IC, (mu + 1) * IC)
        for t in range(2):
            j = mu + 4 * t
            if j > 4:
                continue
            for i in range(5):
                kt8[r, i, t, 0:64] = FP8_SCALE * K[:, :, 0, j, i].T  # edge-lo Ta
                kt8[r, 9 + i, t, 64:128] = FP8_SCALE * K[:, :, 4, j, i].T  # hi Tb
            for pprime in range(1, 5):
                kt8[r, 4 + pprime, t, 0:64] = FP8_SCALE * K[:, :, pprime, j, 4].T
                kt8[r, 4 + pprime, t, 64:128] = (
                    FP8_SCALE * K[:, :, pprime - 1, j, 4].T
                )
    return (
        np.ascontiguousarray(ktm.astype(mnp)),
        np.ascontiguousarray(kt8.reshape(128, N_FP8_BLK * 256).astype(f8np)),
    )


def _kernel_v2(x, weight, P, bias):
    import ml_dtypes

    mnp = ml_dtypes.bfloat16
    f8np = ml_dtypes.float8_e4m3
    K = _construct_K(weight, P)
    ktm_np, kt8_np = _prep_v2_weights(K, mnp, f8np)
    bias_in = np.ascontiguousarray(bias.reshape(OC, 1))

    xpad = np.pad(x, ((0, 0), (0, 0), (PAD, PAD), (PAD, PAD), (PAD, PAD)))
    in_maps = []
    for ci in range(8):
        n, dc = divmod(ci, 4)
        slab = xpad[n, :, 4 * dc : 4 * dc + DSLAB].reshape(IC, SLABF)
        xsb = np.zeros((IC, XSB_COLS), mnp)
        xsb[:, : min(SLABF, XSB_COLS)] = slab[:, :XSB_COLS].astype(mnp)
        xs8 = np.zeros((IC, XS8_COLS), f8np)
        xs8[:, :SLABF] = slab.astype(f8np)
        in_maps.append(
            {"xsb": xsb, "xs8": xs8, "ktm": ktm_np, "kt8": kt8_np, "bias": bias_in}
        )

    global _last_in_maps, _last_mm, _last_build
    _last_in_maps = in_maps
    _last_mm = "bf16"
    _last_build = _build_nc_v2
    nc = _build_nc_v2()
    res = run_bass_kernel_spmd(nc, in_maps, core_ids=list(range(8)))

    out = np.empty((N, OC, D, H, W), np.float32)
    for ci in range(8):
        n, dc = divmod(ci, 4)
        out[n, :, 4 * dc : 4 * dc + DCHUNK] = res.results[ci]["out"].reshape(
            OC, DCHUNK, H, W
        )
    return out


def kernel(x, weight, P, bias, mm="bf16", ver="v2"):
    import ml_dtypes

    x = np.ascontiguousarray(np.asarray(x, dtype=np.float32))
    weight = np.asarray(weight, dtype=np.float32)
    P = np.asarray(P, dtype=np.float32)
    bias = np.asarray(bias, dtype=np.float32)
    if ver == "v2":
        return _kernel_v2(x, weight, P, bias)
    mnp = {"f32": np.float32, "bf16": ml_dtypes.bfloat16}[mm]

    K = _construct_K(weight, P)  # (oc, ic, l, j, i)
    # lhsT layouts: partition=(i, ic), free=(l*5+j slot, oc)
    Kt = K.transpose(4, 1, 2, 3, 0)  # (i, ic, l, j, oc)
    KtF = Kt.reshape(5, IC, NTAPS_LJ, OC)
    bias_in = np.ascontiguousarray(bias.reshape(OC, 1))

    xpad = np.pad(x, ((0, 0), (0, 0), (PAD, PAD), (PAD, PAD), (PAD, PAD)))

    if ver == "v0":
        kt = np.ascontiguousarray(KtF[:4].reshape(128, NTAPS_LJ * OC).astype(mnp))
        kt4 = np.ascontiguousarray(KtF[4].reshape(IC, NTAPS_LJ * OC).astype(mnp))
        extra = {"kt": kt, "kt4": kt4}
        build = _build_nc
    else:
        kta = np.ascontiguousarray(
            KtF[:4][:, :, LJ_A, :].reshape(128, len(LJ_A) * OC).astype(mnp)
        )
        ktb = np.ascontiguousarray(
            KtF[:4][:, :, LJ_B, :].reshape(128, len(LJ_B) * OC).astype(mnp)
        )
        # ktd: partition (l, ic) for l=0..3, free (j, oc): taps (l, j, i=4)
        ktd = np.zeros((128, 5 * OC), mnp)
        for j in range(5):
            for l in range(4):
                ktd[32 * l : 32 * (l + 1), j * OC : (j + 1) * OC] = KtF[
                    4, :, l * 5 + j, :
                ].astype(mnp)
        # ktj: partition (j, ic) for j=0..3: taps (l=4, j, i=4)
        ktj = np.zeros((128, OC), mnp)
        for j in range(4):
            ktj[32 * j : 32 * (j + 1), :] = KtF[4, :, 4 * 5 + j, :].astype(mnp)
        kt5 = np.ascontiguousarray(KtF[4, :, 24, :].astype(mnp))  # (l=4,j=4,i=4)
        extra = {"kta": kta, "ktb": ktb, "ktd": ktd, "ktj": ktj, "kt5": kt5}
        build = _build_nc_packed

    in_maps = []
    for ci in range(8):
        n, dc = divmod(ci, 4)
        slab = xpad[n, :, 4 * dc : 4 * dc + DSLAB].reshape(IC, SLABF)
        xs = np.zeros((IC, XS_COLS), mnp)
        xs[:, :SLABF] = slab.astype(mnp)
        in_maps.append({"xs": xs, "bias": bias_in, **extra})

    global _last_in_maps, _last_mm, _last_build
    _last_in_maps = in_maps
    _last_mm = mm
    _last_build = build
    nc = build(mm)
    res = run_bass_kernel_spmd(nc, in_maps, core_ids=list(range(8)))

    out = np.empty((N, OC, D, H, W), np.float32)
    for ci in range(8):
        n, dc = divmod(ci, 4)
        out[n, :, 4 * dc : 4 * dc + DCHUNK] = res.results[ci]["out"].reshape(
            OC, DCHUNK, H, W
        )
    return out

